# revision 32
# baseline (speedup 1.0000x reference)
"""Multi-head causal attention (B=8, S=1024, C=1024, H=16, dk=dv=64) on 8 trn2 cores.

Sharding: data-parallel over batch. Each NeuronCore processes one batch element
end-to-end (projections + attention + output projection); no collectives.

Per-core design notes (v2 rewrite):
  QT/KT = w.T @ x  -> [H*DK, S] head-pair-major rows (pair hp rows: head 2hp in
    partitions 0:64, head 2hp+1 in 64:128). Evacuated on ACT (Identity+bias).
  V = x.T @ wv -> [S, H*DV] (+ ones column per head for softmax denominators),
    evacuated on DVE with bias add.
  Attention per head pair, per 512-wide q-chunk:
    - score matmuls are K=64 and head-paired: the two heads' matmuls target
      disjoint PE row groups (tile_position via base partition 0/64) and run
      concurrently.
    - causal diag masking is done ON the PE: a tiny N=128 matmul writes -200
      into the strict-upper triangle of each diagonal block before the score
      matmul accumulates on top; exp then gives exact zeros.
    - score blocks for several (head, k-tile) pairs are packed column-wise
      into shared PSUM banks so ONE merged ACT exp instruction covers a whole
      2-bank round (amortizes the ~370ns ACT access overhead).
    - softmax denominator: ones column appended to V (row 64 of the PV
      accumulator); 1/r applied as: copy r row to SBUF, broadcast via a K=1
      f32r matmul (ones64.T @ r) into PSUM, one DVE divide into oT.
  Y = oT.T-contract @ wo + bo -> [S, C] f32, all output-proj m-tiles at end.
"""

import math
import os
import sys

import numpy as np

try:
    import concourse.bass as bass
except ImportError:  # make concourse importable in a bare grading dir
    for _p in ("/opt/trn_rl_repo", os.path.expanduser("~/.axon_site/_ro/trn_rl_repo")):
        if os.path.isdir(_p) and _p not in sys.path:
            sys.path.insert(0, _p)
    import concourse.bass as bass

from contextlib import ExitStack

import ml_dtypes

import concourse.mybir as mybir
import concourse.tile as tile
from concourse import bacc
from concourse.bass_utils import run_bass_kernel_spmd


def _setup_act_tables():
    """Pin the ACT function table to the set that covers exp+ln+identity+copy
    so the kernel never reloads LUTs mid-flight."""
    import json
    import shutil
    import tempfile

    import concourse.hw_specs as hw_specs
    from concourse import bacc as _bacc

    if os.environ.get("BASS_ACT_ROOT_JSON_PATH"):
        return  # already configured
    from neuronxcc.driver.Job import Job

    orig = os.path.join(
        Job.getPackageDir(), "pwp", "pwp_bin_trainium", "act_info.json"
    )
    assert os.path.isfile(orig), orig
    dst = os.path.join(tempfile.gettempdir(), "mha_act_tables")
    if not os.path.isdir(dst):
        tmp = dst + ".tmp"
        shutil.rmtree(tmp, ignore_errors=True)
        shutil.copytree(os.path.dirname(orig), tmp)
        with open(os.path.join(tmp, "act_info.json")) as f:
            info = json.load(f)
        sets = info["act_func_sets"]
        want = [s for s in sets if s["name"] == "natural_log_exp_and_others"]
        rest = [s for s in sets if s["name"] != "natural_log_exp_and_others"]
        info["act_func_sets"] = want + rest
        with open(os.path.join(tmp, "act_info.json"), "w") as f:
            json.dump(info, f)
        os.replace(tmp, dst)
    path = os.path.join(dst, "act_info.json")
    os.environ["BASS_ACT_ROOT_JSON_PATH"] = path

    def patched(module_arch):
        with open(path) as af:
            act_info = json.load(af)
        return {
            ent["name"]: {
                mybir.ActivationFunctionType.from_pwp(v) for v in ent["act"].keys()
            }
            for ent in act_info["act_func_sets"]
        }

    hw_specs.get_activation_tables = patched
    _bacc.get_activation_tables = patched
    from concourse import bass_interp as _bi

    _bi.get_activation_tables = patched


B, S, C = 8, 1024, 1024
H, DK, DV = 16, 64, 64
P = 128
NT = 8  # number of 128-tiles along S / C / H*DK
CH = 512  # free-dim chunk (one PSUM bank of fp32)
NCH = S // CH
NEG = -60.0  # additive causal mask; exp(-60+s) ~ 1e-22, and stays inside
# the range the HW ACT exp table handles (very negative inputs are undefined)

FP = mybir.dt.float32
FR = mybir.dt.float32r
BF = mybir.dt.bfloat16
BF_NP = ml_dtypes.bfloat16
AFT = mybir.ActivationFunctionType
ALU = mybir.AluOpType


def _bank_plan(jc):
    """Column-packed PSUM bank plan for q-chunk jc.

    Returns a list of banks; each bank is a list of entries
    (sub, i, off, w, cs): head sub in {0,1}, k-tile i, valid q-span starts at
    local q-offset `off` (width w), placed at bank column `cs`.
    """
    banks = []
    nfull = 4 * jc + 1  # i in [0, 4*jc] have off == 0
    for i in range(nfull):
        banks.append([(0, i, 0, 512, 0)])
        banks.append([(1, i, 0, 512, 0)])
    i1, i2, i3 = 4 * jc + 1, 4 * jc + 2, 4 * jc + 3
    banks.append([(0, i1, 128, 384, 0), (0, i3, 384, 128, 384)])
    banks.append([(1, i1, 128, 384, 0), (1, i3, 384, 128, 384)])
    banks.append([(0, i2, 256, 256, 0), (1, i2, 256, 256, 256)])
    return banks


def build_nc() -> bass.Bass:
    _setup_act_tables()
    nc = bacc.Bacc()

    xq = nc.dram_tensor("xq", [C, S], BF, kind="ExternalInput")
    xk = nc.dram_tensor("xk", [C, S], BF, kind="ExternalInput")
    xv = nc.dram_tensor("xv", [C, S], BF, kind="ExternalInput")
    wq = nc.dram_tensor("wq", [C, H * DK], BF, kind="ExternalInput")
    wk = nc.dram_tensor("wk", [C, H * DK], BF, kind="ExternalInput")
    wv = nc.dram_tensor("wv", [C, H * DV], BF, kind="ExternalInput")
    wo = nc.dram_tensor("wo", [H * DV, C], BF, kind="ExternalInput")
    bqd = nc.dram_tensor("bq", [P, NT], FP, kind="ExternalInput")
    bkd = nc.dram_tensor("bk", [P, NT], FP, kind="ExternalInput")
    bvd = nc.dram_tensor("bv", [P, H * DV], FP, kind="ExternalInput")
    bod = nc.dram_tensor("bo", [1, C], FP, kind="ExternalInput")
    y = nc.dram_tensor("y", [S, C], FP, kind="ExternalOutput")

    # constants baked into the NEFF
    # row-packed [P, 128+128+64]: I128 | Mneg | I64-stacked (both halves)
    cblk_np = np.zeros((P, 2 * P + DV), np.float32)
    cblk_np[:, 0:P] = np.eye(P, dtype=np.float32)
    cblk_np[:, P : 2 * P] = np.tril(np.full((P, P), NEG, np.float32), -1)
    cblk_np[0:DV, 2 * P : 2 * P + DV] = np.eye(DV, dtype=np.float32)
    cblk_np[DV:P, 2 * P : 2 * P + DV] = np.eye(DV, dtype=np.float32)
    cblk_bf_d = nc.inline_tensor(cblk_np.astype(BF_NP), "cblkb")

    xq_r = xq.rearrange("(ko p) s -> p ko s", p=P)
    xk_r = xk.rearrange("(ko p) s -> p ko s", p=P)
    xv_r = xv.rearrange("(ko p) s -> p ko s", p=P)
    wq_r = wq.rearrange("(ko p) m -> p ko m", p=P)
    wk_r = wk.rearrange("(ko p) m -> p ko m", p=P)
    wv_r = wv.rearrange("(ko p) m -> p ko m", p=P)
    wo_r = wo.rearrange("(ko p) c -> p ko c", p=P)
    y_r = y.rearrange("(mo p) c -> p mo c", p=P)

    with tile.TileContext(nc) as tc, ExitStack() as octx:
        const = octx.enter_context(tc.tile_pool(name="const", bufs=1))
        qk = octx.enter_context(tc.tile_pool(name="qk", bufs=1))
        opool = octx.enter_context(tc.tile_pool(name="oT", bufs=1))
        small = octx.enter_context(tc.tile_pool(name="small", bufs=4))
        ypool = octx.enter_context(tc.tile_pool(name="y", bufs=2))

        # NOTE: DMA issue order matters — the sync queue is serial, so the
        # K-projection inputs are issued first (inside the projection section)
        # and the bulky wo/bv/bo constants are issued after them.
        cbf_sb = const.tile([P, 2 * P + DV], BF, tag="cbf")
        i128_sb = cbf_sb[:, 0:P]
        mneg_sb = cbf_sb[:, P : 2 * P]
        i64s_sb = cbf_sb[:, 2 * P : 2 * P + DV]
        bq_sb = const.tile([P, NT], FP, tag="bq")
        bk_sb = const.tile([P, NT], FP, tag="bk")
        bv_sb = const.tile([P, H * DV], FP, tag="bv")
        bo_sb = const.tile([1, C], FP, tag="bo")
        borep_sb = const.tile([P, C], FP, tag="borep")
        wo_sb = const.tile([P, NT, C], BF, tag="wo")

        def load_consts_early():  # small, needed by first evacs / attention
            nc.sync.dma_start(cbf_sb, cblk_bf_d[:])
            nc.sync.dma_start(bk_sb, bkd[:])
            nc.sync.dma_start(bq_sb, bqd[:])

        def load_consts_late():  # bulky or needed late
            nc.sync.dma_start(bv_sb, bvd[:])
            nc.sync.dma_start(bo_sb, bod[:])
            nc.gpsimd.partition_broadcast(borep_sb, bo_sb)
            nc.sync.dma_start(wo_sb, wo_r)

        qT_sb = qk.tile([P, NT, S], BF, tag="qT")
        kT_sb = qk.tile([P, NT, S], BF, tag="kT")
        v_sb = qk.tile([P, NT, H, DV + 1], BF, tag="v")
        oT_sb = opool.tile([P, NT, S], BF, tag="oT")

        nc.vector.memset(v_sb[:, :, :, DV], 1.0)

        # ---------------- projections ----------------
        with ExitStack() as ictx:
            wpool = ictx.enter_context(tc.tile_pool(name="wqkv", bufs=2))
            xpool = ictx.enter_context(tc.tile_pool(name="xin", bufs=2))
            psproj = ictx.enter_context(
                tc.tile_pool(name="psproj", bufs=8, space="PSUM")
            )

            def proj_half(w_sb, x_sb, ms, emit_evac, tag):
                """One half (4 m-tiles) of a projection, kc-outer so compute
                chases the per-kc DMAs. Each K=128 contraction step is split
                into two concurrent K=64 row-strip matmuls (disjoint PE row
                groups -> LDWEIGHTS of one strip hides under the other's
                stream)."""
                ps = {
                    (m, n): psproj.tile(
                        [P, CH], FP, tag="proj", name=f"pp_{tag}_{m}_{n}"
                    )
                    for m in ms
                    for n in range(NCH)
                }
                for kc in range(NT):
                    for m in ms:
                        for n in range(NCH):
                            nc.tensor.matmul(
                                ps[(m, n)],
                                w_sb[:, kc, m * P : (m + 1) * P],
                                x_sb[:, kc, n * CH : (n + 1) * CH],
                                start=(kc == 0),
                                stop=(kc == NT - 1),
                            )
                for m in ms:
                    for n in range(NCH):
                        emit_evac(m, n, ps[(m, n)])

            # K^T and Q^T: out[hk, s]; lhsT = w tile [c, hk], rhs = x^T [c, s]
            # evacuated on ACT with per-partition bias. n-inner for LDW reuse.
            for pi, (x_r, w_r, b_sb, out_sb) in enumerate(
                (
                    (xk_r, wk_r, bk_sb, kT_sb),
                    (xq_r, wq_r, bq_sb, qT_sb),
                )
            ):
                w_sb = wpool.tile([P, NT, H * DK], BF, tag="w", name="w_sb")
                x_sb = xpool.tile([P, NT, S], BF, tag="x", name="x_sb")
                for kc in range(NT):
                    nc.sync.dma_start(w_sb[:, kc], w_r[:, kc])
                    nc.sync.dma_start(x_sb[:, kc], x_r[:, kc])
                if pi == 0:
                    load_consts_early()

                def qk_evac(m, n, psum, out_sb=out_sb, b_sb=b_sb):
                    nc.scalar.activation(
                        out_sb[:, m, n * CH : (n + 1) * CH],
                        psum,
                        AFT.Identity,
                        bias=b_sb[:, m : m + 1],
                    )

                proj_half(w_sb, x_sb, range(0, 4), qk_evac, f"qk{pi}a")
                proj_half(w_sb, x_sb, range(4, NT), qk_evac, f"qk{pi}b")

            # V: out[s, hv]; lhsT = x^T tile [c, s], rhs = wv [c, hv]
            wv_sb = wpool.tile([P, NT, H * DV], BF, tag="w", name="wv_sb")
            xv_sb = xpool.tile([P, NT, S], BF, tag="x", name="xv_sb")
            for kc in range(NT):
                nc.sync.dma_start(wv_sb[:, kc], wv_r[:, kc])
                nc.sync.dma_start(xv_sb[:, kc], xv_r[:, kc])
            load_consts_late()

            def v_evac(m, n, psum):
                nc.vector.tensor_tensor(
                    v_sb[:, m, 8 * n : 8 * (n + 1), 0:DV],
                    psum.rearrange("p (h v) -> p h v", v=DV),
                    bv_sb[:, n * CH : (n + 1) * CH].rearrange(
                        "p (h v) -> p h v", v=DV
                    ),
                    ALU.add,
                )

            proj_half(xv_sb, wv_sb, range(0, 4), v_evac, "va")
            proj_half(xv_sb, wv_sb, range(4, NT), v_evac, "vb")

        # ---------------- attention ----------------
        actx = octx.enter_context(ExitStack())
        ps_st = actx.enter_context(tc.tile_pool(name="ps_st", bufs=2, space="PSUM"))
        ps_pv = actx.enter_context(tc.tile_pool(name="ps_pv", bufs=2, space="PSUM"))
        ps_aux = actx.enter_context(tc.tile_pool(name="ps_aux", bufs=2, space="PSUM"))
        ppool = actx.enter_context(tc.tile_pool(name="pch", bufs=3))
        oupool = actx.enter_context(tc.tile_pool(name="ou", bufs=3))

        def attn_pair(hp, jc):
            banks = _bank_plan(jc)
            nblk = 4 * (jc + 1)  # PV blocks per head
            jq = jc * CH
            pos = {
                s: ps_pv.tile([P, CH], FP, tag="pv", name=f"pv_{hp}_{jc}_{s}")[
                    : DV + 1
                ]
                for s in (0, 1)
            }
            pv_count = {0: 0, 1: 0}
            pending = []  # (bank-entries, pch tile, bank-in-tile idx)

            def flush_pv():
                for ents, pch, b in pending:
                    for (s, i, off, w, cs) in ents:
                        h = 2 * hp + s
                        nc.tensor.matmul(
                            pos[s][:, off : off + w],
                            v_sb[:, i, h, :],
                            pch[:, b, cs : cs + w],
                            start=(pv_count[s] == 0),
                            stop=(pv_count[s] == nblk - 1),
                            skip_group_check=True,
                        )
                        pv_count[s] += 1
                pending.clear()

            for r0 in range(0, len(banks), 2):
                rbanks = banks[r0 : r0 + 2]
                nb = len(rbanks)
                pst = ps_st.tile([P, 2, CH], FP, tag="st", name=f"st_{hp}_{jc}_{r0}")
                # Per bank entry: score matmul first (start=True on the bank's
                # first — marks the bank pending-zero so later disjoint spans
                # overwrite), then the diag-mask matmul accumulates -200 onto
                # the already-written 128 diag columns. Banks are interleaved
                # so head-paired K=64 score matmuls on disjoint row groups run
                # concurrently.
                nmm = [
                    sum(1 + (i >= 4 * jc) for (s, i, off, w, cs) in ents)
                    for ents in rbanks
                ]
                done = [0] * nb
                maxe = max(len(e) for e in rbanks)
                for e in range(maxe):
                    for b, ents in enumerate(rbanks):
                        if e >= len(ents):
                            continue
                        (s, i, off, w, cs) = ents[e]
                        p0 = s * DV
                        nc.tensor.matmul(
                            pst[:, b, cs : cs + w],
                            kT_sb[p0 : p0 + DK, hp, i * P : (i + 1) * P],
                            qT_sb[p0 : p0 + DK, hp, jq + off : jq + CH],
                            start=(done[b] == 0),
                            stop=(done[b] == nmm[b] - 1),
                            skip_group_check=True,
                        )
                        done[b] += 1
                    for b, ents in enumerate(rbanks):
                        if e >= len(ents):
                            continue
                        (s, i, off, w, cs) = ents[e]
                        if i >= 4 * jc:  # diagonal-crossing block
                            nc.tensor.matmul(
                                pst[:, b, cs : cs + P],
                                i128_sb,
                                mneg_sb,
                                start=False,
                                stop=(done[b] == nmm[b] - 1),
                                skip_group_check=True,
                            )
                            done[b] += 1
                # PV of the previous round goes to the PE queue after this
                # round's scores so the PE never stalls waiting on exp.
                flush_pv()
                pch = ppool.tile(
                    [P, 2, CH], BF, tag="p", name=f"p_{hp}_{jc}_{r0}"
                )
                nc.scalar.activation(pch[:, :nb], pst[:, :nb], AFT.Exp)
                pending.extend(
                    (ents, pch, b) for b, ents in enumerate(rbanks)
                )
            flush_pv()

            for s in (0, 1):
                hm = s * DV
                # evacuate the accumulator at once so the PSUM bank recycles
                # immediately; the 1/r chain then runs off the critical path
                ou = oupool.tile([DV + 1, CH], FP, tag="ou", name=f"ou_{hp}_{jc}_{s}")
                nc.vector.tensor_copy(out=ou, in_=pos[s])
                # NB: reciprocal_approx_fast (custom DVE op) requires its
                # input AP to start at partition 0 — stage the r row there.
                r0_sb = small.tile([1, CH], FP, tag="r0", name=f"r0_{hp}_{jc}_{s}")
                nc.vector.tensor_copy(out=r0_sb, in_=ou[DV : DV + 1, :])
                ri_sb = small.tile([1, CH], FP, tag="r", name=f"ri_{hp}_{jc}_{s}")
                nc.vector.reciprocal_approx_fast(ri_sb, r0_sb)
                rrep = small.tile([DV, CH], FP, tag="rr", name=f"rr_{hp}_{jc}_{s}")
                nc.gpsimd.partition_broadcast(rrep, ri_sb)
                nc.vector.tensor_tensor(
                    oT_sb[hm : hm + DV, hp, jq : jq + CH],
                    ou[0:DV],
                    rrep,
                    ALU.mult,
                )

        def outproj_mtile(m, pool):
            ps = [
                pool.tile([P, CH], FP, tag="y", name=f"py_{m}_{n}")
                for n in range(NCH)
            ]
            for kc in range(NT):
                for n in range(NCH):
                    nc.tensor.matmul(
                        ps[n],
                        oT_sb[:, kc, m * P : (m + 1) * P],
                        wo_sb[:, kc, n * CH : (n + 1) * CH],
                        start=(kc == 0),
                        stop=(kc == NT - 1),
                    )
            for n in range(NCH):
                yt = ypool.tile([P, CH], FP, tag="y", name=f"yt_{m}_{n}")
                nc.vector.tensor_tensor(
                    yt, ps[n], borep_sb[:, n * CH : (n + 1) * CH], ALU.add
                )
                nc.sync.dma_start(y_r[:, m, n * CH : (n + 1) * CH], yt)

        for hp in range(H // 2):
            attn_pair(hp, 0)
        for hp in range(H // 2):
            attn_pair(hp, 1)
            if hp % 2 == 1:
                # y rows [0, 512) depend only on jc=0 (done); interleave their
                # output projection into the ACT-bound jc=1 stretch
                outproj_mtile(hp // 2, ps_aux)
        actx.close()

        # ---------------- output projection (second q-chunk) ----------------
        with ExitStack() as fctx:
            psy = fctx.enter_context(tc.tile_pool(name="psy", bufs=4, space="PSUM"))
            for m in range(4, NT):
                outproj_mtile(m, psy)

    nc.finalize()
    return nc


_NC_CACHE = None


def _get_nc() -> bass.Bass:
    global _NC_CACHE
    if _NC_CACHE is None:
        _NC_CACHE = build_nc()
    return _NC_CACHE


def prep_shared(Wq, bq, Wk, bk, Wv, bv, Wo, bo):
    """Host-side packing of weights/biases (shared by all cores)."""
    scale = 1.0 / math.sqrt(DK)
    Wq = np.asarray(Wq, np.float32)
    Wk = np.asarray(Wk, np.float32)
    Wv = np.asarray(Wv, np.float32)
    Wo = np.asarray(Wo, np.float32)
    out = {
        "wq": np.ascontiguousarray(
            (Wq.transpose(1, 0, 2).reshape(C, H * DK) * scale).astype(BF_NP)
        ),
        "wk": np.ascontiguousarray(
            Wk.transpose(1, 0, 2).reshape(C, H * DK).astype(BF_NP)
        ),
        "wv": np.ascontiguousarray(
            Wv.transpose(1, 0, 2).reshape(C, H * DV).astype(BF_NP)
        ),
        "wo": Wo.astype(BF_NP),
        "bq": np.ascontiguousarray(
            (np.asarray(bq, np.float32).reshape(H * DK) * scale)
            .reshape(NT, P)
            .T.astype(np.float32)
        ),
        "bk": np.ascontiguousarray(
            np.asarray(bk, np.float32).reshape(NT, P).T.astype(np.float32)
        ),
        "bv": np.ascontiguousarray(
            np.broadcast_to(
                np.asarray(bv, np.float32).reshape(1, H * DV), (P, H * DV)
            ).astype(np.float32)
        ),
        "bo": np.ascontiguousarray(np.asarray(bo, np.float32).reshape(1, C)),
    }
    return out


def prep_core(q_embs_b, k_embs_b, v_embs_b):
    return {
        "xq": np.ascontiguousarray(np.asarray(q_embs_b, np.float32).T.astype(BF_NP)),
        "xk": np.ascontiguousarray(np.asarray(k_embs_b, np.float32).T.astype(BF_NP)),
        "xv": np.ascontiguousarray(np.asarray(v_embs_b, np.float32).T.astype(BF_NP)),
    }


def kernel(q_embs, k_embs, v_embs, Wq, bq, Wk, bk, Wv, bv, Wo, bo, **run_kwargs):
    nc = _get_nc()
    shared = prep_shared(Wq, bq, Wk, bk, Wv, bv, Wo, bo)
    q_embs = np.asarray(q_embs, np.float32)
    k_embs = np.asarray(k_embs, np.float32)
    v_embs = np.asarray(v_embs, np.float32)
    in_maps = []
    for b in range(B):
        m = dict(shared)
        m.update(prep_core(q_embs[b], k_embs[b], v_embs[b]))
        in_maps.append(m)
    res = run_bass_kernel_spmd(nc, in_maps, core_ids=list(range(B)), **run_kwargs)
    out = np.stack([res.results[i]["y"] for i in range(B)], axis=0)
    if run_kwargs:
        kernel.last_results = res
    return out


if __name__ == "__main__":
    rng = np.random.default_rng(0)
    inputs = {
        "q_embs": rng.standard_normal((B, S, C), np.float32),
        "k_embs": rng.standard_normal((B, S, C), np.float32),
        "v_embs": rng.standard_normal((B, S, C), np.float32),
        "Wq": rng.standard_normal((H, C, DK), np.float32) * 0.02,
        "bq": np.zeros((H, DK), np.float32),
        "Wk": rng.standard_normal((H, C, DK), np.float32) * 0.02,
        "bk": np.zeros((H, DK), np.float32),
        "Wv": rng.standard_normal((H, C, DV), np.float32) * 0.02,
        "bv": np.zeros((H, DV), np.float32),
        "Wo": rng.standard_normal((H * DV, C), np.float32) * 0.02,
        "bo": np.zeros((C,), np.float32),
    }
    out = kernel(**inputs)
    print(out.shape, out.dtype)


# revision 33
# speedup vs baseline: 1.0097x; 1.0097x over previous
"""Multi-head causal attention (B=8, S=1024, C=1024, H=16, dk=dv=64) on 8 trn2 cores.

Sharding: data-parallel over batch. Each NeuronCore processes one batch element
end-to-end (projections + attention + output projection); no collectives.

Per-core design notes (v2 rewrite):
  QT/KT = w.T @ x  -> [H*DK, S] head-pair-major rows (pair hp rows: head 2hp in
    partitions 0:64, head 2hp+1 in 64:128). Evacuated on ACT (Identity+bias).
  V = x.T @ wv -> [S, H*DV] (+ ones column per head for softmax denominators),
    evacuated on DVE with bias add.
  Attention per head pair, per 512-wide q-chunk:
    - score matmuls are K=64 and head-paired: the two heads' matmuls target
      disjoint PE row groups (tile_position via base partition 0/64) and run
      concurrently.
    - causal diag masking is done ON the PE: a tiny N=128 matmul writes -200
      into the strict-upper triangle of each diagonal block before the score
      matmul accumulates on top; exp then gives exact zeros.
    - score blocks for several (head, k-tile) pairs are packed column-wise
      into shared PSUM banks so ONE merged ACT exp instruction covers a whole
      2-bank round (amortizes the ~370ns ACT access overhead).
    - softmax denominator: ones column appended to V (row 64 of the PV
      accumulator); 1/r applied as: copy r row to SBUF, broadcast via a K=1
      f32r matmul (ones64.T @ r) into PSUM, one DVE divide into oT.
  Y = oT.T-contract @ wo + bo -> [S, C] f32, all output-proj m-tiles at end.
"""

import math
import os
import sys

import numpy as np

try:
    import concourse.bass as bass
except ImportError:  # make concourse importable in a bare grading dir
    for _p in ("/opt/trn_rl_repo", os.path.expanduser("~/.axon_site/_ro/trn_rl_repo")):
        if os.path.isdir(_p) and _p not in sys.path:
            sys.path.insert(0, _p)
    import concourse.bass as bass

from contextlib import ExitStack

import ml_dtypes

import concourse.mybir as mybir
import concourse.tile as tile
from concourse import bacc
from concourse.bass_utils import run_bass_kernel_spmd


def _setup_act_tables():
    """Pin the ACT function table to the set that covers exp+ln+identity+copy
    so the kernel never reloads LUTs mid-flight."""
    import json
    import shutil
    import tempfile

    import concourse.hw_specs as hw_specs
    from concourse import bacc as _bacc

    if os.environ.get("BASS_ACT_ROOT_JSON_PATH"):
        return  # already configured
    from neuronxcc.driver.Job import Job

    orig = os.path.join(
        Job.getPackageDir(), "pwp", "pwp_bin_trainium", "act_info.json"
    )
    assert os.path.isfile(orig), orig
    dst = os.path.join(tempfile.gettempdir(), "mha_act_tables")
    if not os.path.isdir(dst):
        tmp = dst + ".tmp"
        shutil.rmtree(tmp, ignore_errors=True)
        shutil.copytree(os.path.dirname(orig), tmp)
        with open(os.path.join(tmp, "act_info.json")) as f:
            info = json.load(f)
        sets = info["act_func_sets"]
        want = [s for s in sets if s["name"] == "natural_log_exp_and_others"]
        rest = [s for s in sets if s["name"] != "natural_log_exp_and_others"]
        info["act_func_sets"] = want + rest
        with open(os.path.join(tmp, "act_info.json"), "w") as f:
            json.dump(info, f)
        os.replace(tmp, dst)
    path = os.path.join(dst, "act_info.json")
    os.environ["BASS_ACT_ROOT_JSON_PATH"] = path

    def patched(module_arch):
        with open(path) as af:
            act_info = json.load(af)
        return {
            ent["name"]: {
                mybir.ActivationFunctionType.from_pwp(v) for v in ent["act"].keys()
            }
            for ent in act_info["act_func_sets"]
        }

    hw_specs.get_activation_tables = patched
    _bacc.get_activation_tables = patched
    from concourse import bass_interp as _bi

    _bi.get_activation_tables = patched


B, S, C = 8, 1024, 1024
H, DK, DV = 16, 64, 64
P = 128
NT = 8  # number of 128-tiles along S / C / H*DK
CH = 512  # free-dim chunk (one PSUM bank of fp32)
NCH = S // CH
NEG = -60.0  # additive causal mask; exp(-60+s) ~ 1e-22, and stays inside
# the range the HW ACT exp table handles (very negative inputs are undefined)

FP = mybir.dt.float32
FR = mybir.dt.float32r
BF = mybir.dt.bfloat16
BF_NP = ml_dtypes.bfloat16
AFT = mybir.ActivationFunctionType
ALU = mybir.AluOpType


def _bank_plan(jc):
    """Column-packed PSUM bank plan for q-chunk jc.

    Returns a list of banks; each bank is a list of entries
    (sub, i, off, w, cs): head sub in {0,1}, k-tile i, valid q-span starts at
    local q-offset `off` (width w), placed at bank column `cs`.
    """
    banks = []
    nfull = 4 * jc + 1  # i in [0, 4*jc] have off == 0
    for i in range(nfull):
        banks.append([(0, i, 0, 512, 0)])
        banks.append([(1, i, 0, 512, 0)])
    i1, i2, i3 = 4 * jc + 1, 4 * jc + 2, 4 * jc + 3
    banks.append([(0, i1, 128, 384, 0), (0, i3, 384, 128, 384)])
    banks.append([(1, i1, 128, 384, 0), (1, i3, 384, 128, 384)])
    banks.append([(0, i2, 256, 256, 0), (1, i2, 256, 256, 256)])
    return banks


def build_nc() -> bass.Bass:
    _setup_act_tables()
    nc = bacc.Bacc()

    xq = nc.dram_tensor("xq", [C, S], BF, kind="ExternalInput")
    xk = nc.dram_tensor("xk", [C, S], BF, kind="ExternalInput")
    xv = nc.dram_tensor("xv", [C, S], BF, kind="ExternalInput")
    wq = nc.dram_tensor("wq", [C, H * DK], BF, kind="ExternalInput")
    wk = nc.dram_tensor("wk", [C, H * DK], BF, kind="ExternalInput")
    wv = nc.dram_tensor("wv", [C, H * DV], BF, kind="ExternalInput")
    wo = nc.dram_tensor("wo", [H * DV, C], BF, kind="ExternalInput")
    bqd = nc.dram_tensor("bq", [P, NT], FP, kind="ExternalInput")
    bkd = nc.dram_tensor("bk", [P, NT], FP, kind="ExternalInput")
    bvd = nc.dram_tensor("bv", [P, H * DV], FP, kind="ExternalInput")
    bod = nc.dram_tensor("bo", [1, C], FP, kind="ExternalInput")
    y = nc.dram_tensor("y", [S, C], FP, kind="ExternalOutput")

    # constants baked into the NEFF
    # row-packed [P, 128+128+64]: I128 | Mneg | (row0: ones64)
    cblk_np = np.zeros((P, 2 * P + DV), np.float32)
    cblk_np[:, 0:P] = np.eye(P, dtype=np.float32)
    cblk_np[:, P : 2 * P] = np.tril(np.full((P, P), NEG, np.float32), -1)
    cblk_np[0, 2 * P : 2 * P + DV] = 1.0
    cblk_bf_d = nc.inline_tensor(cblk_np[:, : 2 * P].astype(BF_NP), "cblkb")

    xq_r = xq.rearrange("(ko p) s -> p ko s", p=P)
    xk_r = xk.rearrange("(ko p) s -> p ko s", p=P)
    xv_r = xv.rearrange("(ko p) s -> p ko s", p=P)
    wq_r = wq.rearrange("(ko p) m -> p ko m", p=P)
    wk_r = wk.rearrange("(ko p) m -> p ko m", p=P)
    wv_r = wv.rearrange("(ko p) m -> p ko m", p=P)
    wo_r = wo.rearrange("(ko p) c -> p ko c", p=P)
    y_r = y.rearrange("(mo p) c -> p mo c", p=P)

    with tile.TileContext(nc) as tc, ExitStack() as octx:
        const = octx.enter_context(tc.tile_pool(name="const", bufs=1))
        qk = octx.enter_context(tc.tile_pool(name="qk", bufs=1))
        opool = octx.enter_context(tc.tile_pool(name="oT", bufs=1))
        small = octx.enter_context(tc.tile_pool(name="small", bufs=4))
        ypool = octx.enter_context(tc.tile_pool(name="y", bufs=2))

        # NOTE: DMA issue order matters — the sync queue is serial, so the
        # K-projection inputs are issued first (inside the projection section)
        # and the bulky wo/bv/bo constants are issued after them.
        cbf_sb = const.tile([P, 2 * P], BF, tag="cbf")
        i128_sb = cbf_sb[:, 0:P]
        mneg_sb = cbf_sb[:, P : 2 * P]
        bq_sb = const.tile([P, NT], FP, tag="bq")
        bk_sb = const.tile([P, NT], FP, tag="bk")
        bv_sb = const.tile([P, H * DV], FP, tag="bv")
        bo_sb = const.tile([1, C], FP, tag="bo")
        borep_sb = const.tile([P, C], FP, tag="borep")
        wo_sb = const.tile([P, NT, C], BF, tag="wo")

        def load_consts_early():  # small, needed by first evacs / attention
            nc.sync.dma_start(cbf_sb, cblk_bf_d[:])
            nc.sync.dma_start(bk_sb, bkd[:])
            nc.sync.dma_start(bq_sb, bqd[:])

        def load_consts_late():  # bulky or needed late
            nc.sync.dma_start(bv_sb, bvd[:])
            nc.sync.dma_start(bo_sb, bod[:])
            nc.gpsimd.partition_broadcast(borep_sb, bo_sb)
            nc.sync.dma_start(wo_sb, wo_r)

        qT_sb = qk.tile([P, NT, S], BF, tag="qT")
        kT_sb = qk.tile([P, NT, S], BF, tag="kT")
        v_sb = qk.tile([P, NT, H, DV + 1], BF, tag="v")
        oT_sb = opool.tile([P, NT, S], BF, tag="oT")

        nc.vector.memset(v_sb[:, :, :, DV], 1.0)

        # ---------------- projections ----------------
        with ExitStack() as ictx:
            wpool = ictx.enter_context(tc.tile_pool(name="wqkv", bufs=2))
            xpool = ictx.enter_context(tc.tile_pool(name="xin", bufs=2))
            psproj = ictx.enter_context(
                tc.tile_pool(name="psproj", bufs=4, space="PSUM")
            )

            # K^T and Q^T: out[hk, s]; lhsT = w tile [c, hk], rhs = x^T [c, s]
            # evacuated on ACT with per-partition bias. n-inner for LDW reuse.
            for pi, (x_r, w_r, b_sb, out_sb) in enumerate(
                (
                    (xk_r, wk_r, bk_sb, kT_sb),
                    (xq_r, wq_r, bq_sb, qT_sb),
                )
            ):
                w_sb = wpool.tile([P, NT, H * DK], BF, tag="w", name="w_sb")
                x_sb = xpool.tile([P, NT, S], BF, tag="x", name="x_sb")
                for kc in range(NT):
                    nc.sync.dma_start(w_sb[:, kc], w_r[:, kc])
                    nc.sync.dma_start(x_sb[:, kc], x_r[:, kc])
                if pi == 0:
                    load_consts_early()
                for m in range(NT):
                    ps = [
                        psproj.tile([P, CH], FP, tag="proj", name=f"pp_{m}_{n}")
                        for n in range(NCH)
                    ]
                    for kc in range(NT):
                        for n in range(NCH):
                            nc.tensor.matmul(
                                ps[n],
                                w_sb[:, kc, m * P : (m + 1) * P],
                                x_sb[:, kc, n * CH : (n + 1) * CH],
                                start=(kc == 0),
                                stop=(kc == NT - 1),
                            )
                    for n in range(NCH):
                        nc.scalar.activation(
                            out_sb[:, m, n * CH : (n + 1) * CH],
                            ps[n],
                            AFT.Identity,
                            bias=b_sb[:, m : m + 1],
                        )

            # V: out[s, hv]; lhsT = x^T tile [c, s], rhs = wv [c, hv]
            wv_sb = wpool.tile([P, NT, H * DV], BF, tag="w", name="wv_sb")
            xv_sb = xpool.tile([P, NT, S], BF, tag="x", name="xv_sb")
            for kc in range(NT):
                nc.sync.dma_start(wv_sb[:, kc], wv_r[:, kc])
                nc.sync.dma_start(xv_sb[:, kc], xv_r[:, kc])
            load_consts_late()
            for m in range(NT):
                ps = [
                    psproj.tile([P, CH], FP, tag="proj", name=f"pv_{m}_{n}")
                    for n in range(NCH)
                ]
                for kc in range(NT):
                    for n in range(NCH):
                        nc.tensor.matmul(
                            ps[n],
                            xv_sb[:, kc, m * P : (m + 1) * P],
                            wv_sb[:, kc, n * CH : (n + 1) * CH],
                            start=(kc == 0),
                            stop=(kc == NT - 1),
                        )
                for n in range(NCH):
                    nc.vector.tensor_tensor(
                        v_sb[:, m, 8 * n : 8 * (n + 1), 0:DV],
                        ps[n].rearrange("p (h v) -> p h v", v=DV),
                        bv_sb[:, n * CH : (n + 1) * CH].rearrange(
                            "p (h v) -> p h v", v=DV
                        ),
                        ALU.add,
                    )

        # ---------------- attention ----------------
        actx = octx.enter_context(ExitStack())
        ps_st = actx.enter_context(tc.tile_pool(name="ps_st", bufs=2, space="PSUM"))
        ps_pv = actx.enter_context(tc.tile_pool(name="ps_pv", bufs=2, space="PSUM"))
        ps_aux = actx.enter_context(tc.tile_pool(name="ps_aux", bufs=2, space="PSUM"))
        ppool = actx.enter_context(tc.tile_pool(name="pch", bufs=3))
        oupool = actx.enter_context(tc.tile_pool(name="ou", bufs=3))

        def attn_pair(hp, jc):
            banks = _bank_plan(jc)
            nblk = 4 * (jc + 1)  # PV blocks per head
            jq = jc * CH
            pos = {
                s: ps_pv.tile([P, CH], FP, tag="pv", name=f"pv_{hp}_{jc}_{s}")[
                    : DV + 1
                ]
                for s in (0, 1)
            }
            pv_count = {0: 0, 1: 0}
            pending = []  # (bank-entries, pch tile, bank-in-tile idx)

            def flush_pv():
                for ents, pch, b in pending:
                    for (s, i, off, w, cs) in ents:
                        h = 2 * hp + s
                        nc.tensor.matmul(
                            pos[s][:, off : off + w],
                            v_sb[:, i, h, :],
                            pch[:, b, cs : cs + w],
                            start=(pv_count[s] == 0),
                            stop=(pv_count[s] == nblk - 1),
                            skip_group_check=True,
                        )
                        pv_count[s] += 1
                pending.clear()

            for r0 in range(0, len(banks), 2):
                rbanks = banks[r0 : r0 + 2]
                nb = len(rbanks)
                pst = ps_st.tile([P, 2, CH], FP, tag="st", name=f"st_{hp}_{jc}_{r0}")
                # Per bank entry: score matmul first (start=True on the bank's
                # first — marks the bank pending-zero so later disjoint spans
                # overwrite), then the diag-mask matmul accumulates -200 onto
                # the already-written 128 diag columns. Banks are interleaved
                # so head-paired K=64 score matmuls on disjoint row groups run
                # concurrently.
                nmm = [
                    sum(1 + (i >= 4 * jc) for (s, i, off, w, cs) in ents)
                    for ents in rbanks
                ]
                done = [0] * nb
                maxe = max(len(e) for e in rbanks)
                for e in range(maxe):
                    for b, ents in enumerate(rbanks):
                        if e >= len(ents):
                            continue
                        (s, i, off, w, cs) = ents[e]
                        p0 = s * DV
                        nc.tensor.matmul(
                            pst[:, b, cs : cs + w],
                            kT_sb[p0 : p0 + DK, hp, i * P : (i + 1) * P],
                            qT_sb[p0 : p0 + DK, hp, jq + off : jq + CH],
                            start=(done[b] == 0),
                            stop=(done[b] == nmm[b] - 1),
                            skip_group_check=True,
                        )
                        done[b] += 1
                    for b, ents in enumerate(rbanks):
                        if e >= len(ents):
                            continue
                        (s, i, off, w, cs) = ents[e]
                        if i >= 4 * jc:  # diagonal-crossing block
                            nc.tensor.matmul(
                                pst[:, b, cs : cs + P],
                                i128_sb,
                                mneg_sb,
                                start=False,
                                stop=(done[b] == nmm[b] - 1),
                                skip_group_check=True,
                            )
                            done[b] += 1
                # PV of the previous round goes to the PE queue after this
                # round's scores so the PE never stalls waiting on exp.
                flush_pv()
                pch = ppool.tile(
                    [P, 2, CH], BF, tag="p", name=f"p_{hp}_{jc}_{r0}"
                )
                nc.scalar.activation(pch[:, :nb], pst[:, :nb], AFT.Exp)
                pending.extend(
                    (ents, pch, b) for b, ents in enumerate(rbanks)
                )
            flush_pv()

            for s in (0, 1):
                hm = s * DV
                # evacuate the accumulator at once so the PSUM bank recycles
                # immediately; the 1/r chain then runs off the critical path
                ou = oupool.tile([DV + 1, CH], FP, tag="ou", name=f"ou_{hp}_{jc}_{s}")
                nc.vector.tensor_copy(out=ou, in_=pos[s])
                # NB: reciprocal_approx_fast (custom DVE op) requires its
                # input AP to start at partition 0 — stage the r row there.
                r0_sb = small.tile([1, CH], FP, tag="r0", name=f"r0_{hp}_{jc}_{s}")
                nc.vector.tensor_copy(out=r0_sb, in_=ou[DV : DV + 1, :])
                ri_sb = small.tile([1, CH], FP, tag="r", name=f"ri_{hp}_{jc}_{s}")
                nc.vector.reciprocal_approx_fast(ri_sb, r0_sb)
                rrep = small.tile([DV, CH], FP, tag="rr", name=f"rr_{hp}_{jc}_{s}")
                nc.gpsimd.partition_broadcast(rrep, ri_sb)
                nc.vector.tensor_tensor(
                    oT_sb[hm : hm + DV, hp, jq : jq + CH],
                    ou[0:DV],
                    rrep,
                    ALU.mult,
                )

        def outproj_mtile(m, pool):
            ps = [
                pool.tile([P, CH], FP, tag="y", name=f"py_{m}_{n}")
                for n in range(NCH)
            ]
            for kc in range(NT):
                for n in range(NCH):
                    nc.tensor.matmul(
                        ps[n],
                        oT_sb[:, kc, m * P : (m + 1) * P],
                        wo_sb[:, kc, n * CH : (n + 1) * CH],
                        start=(kc == 0),
                        stop=(kc == NT - 1),
                    )
            for n in range(NCH):
                yt = ypool.tile([P, CH], FP, tag="y", name=f"yt_{m}_{n}")
                nc.vector.tensor_tensor(
                    yt, ps[n], borep_sb[:, n * CH : (n + 1) * CH], ALU.add
                )
                nc.sync.dma_start(y_r[:, m, n * CH : (n + 1) * CH], yt)

        for hp in range(H // 2):
            attn_pair(hp, 0)
        for hp in range(H // 2):
            attn_pair(hp, 1)
            if hp % 2 == 1:
                # y rows [0, 512) depend only on jc=0 (done); interleave their
                # output projection into the ACT-bound jc=1 stretch
                outproj_mtile(hp // 2, ps_aux)
        actx.close()

        # ---------------- output projection (second q-chunk) ----------------
        with ExitStack() as fctx:
            psy = fctx.enter_context(tc.tile_pool(name="psy", bufs=4, space="PSUM"))
            for m in range(4, NT):
                outproj_mtile(m, psy)

    nc.finalize()
    return nc


_NC_CACHE = None


def _get_nc() -> bass.Bass:
    global _NC_CACHE
    if _NC_CACHE is None:
        _NC_CACHE = build_nc()
    return _NC_CACHE


def prep_shared(Wq, bq, Wk, bk, Wv, bv, Wo, bo):
    """Host-side packing of weights/biases (shared by all cores)."""
    scale = 1.0 / math.sqrt(DK)
    Wq = np.asarray(Wq, np.float32)
    Wk = np.asarray(Wk, np.float32)
    Wv = np.asarray(Wv, np.float32)
    Wo = np.asarray(Wo, np.float32)
    out = {
        "wq": np.ascontiguousarray(
            (Wq.transpose(1, 0, 2).reshape(C, H * DK) * scale).astype(BF_NP)
        ),
        "wk": np.ascontiguousarray(
            Wk.transpose(1, 0, 2).reshape(C, H * DK).astype(BF_NP)
        ),
        "wv": np.ascontiguousarray(
            Wv.transpose(1, 0, 2).reshape(C, H * DV).astype(BF_NP)
        ),
        "wo": Wo.astype(BF_NP),
        "bq": np.ascontiguousarray(
            (np.asarray(bq, np.float32).reshape(H * DK) * scale)
            .reshape(NT, P)
            .T.astype(np.float32)
        ),
        "bk": np.ascontiguousarray(
            np.asarray(bk, np.float32).reshape(NT, P).T.astype(np.float32)
        ),
        "bv": np.ascontiguousarray(
            np.broadcast_to(
                np.asarray(bv, np.float32).reshape(1, H * DV), (P, H * DV)
            ).astype(np.float32)
        ),
        "bo": np.ascontiguousarray(np.asarray(bo, np.float32).reshape(1, C)),
    }
    return out


def prep_core(q_embs_b, k_embs_b, v_embs_b):
    return {
        "xq": np.ascontiguousarray(np.asarray(q_embs_b, np.float32).T.astype(BF_NP)),
        "xk": np.ascontiguousarray(np.asarray(k_embs_b, np.float32).T.astype(BF_NP)),
        "xv": np.ascontiguousarray(np.asarray(v_embs_b, np.float32).T.astype(BF_NP)),
    }


def kernel(q_embs, k_embs, v_embs, Wq, bq, Wk, bk, Wv, bv, Wo, bo, **run_kwargs):
    nc = _get_nc()
    shared = prep_shared(Wq, bq, Wk, bk, Wv, bv, Wo, bo)
    q_embs = np.asarray(q_embs, np.float32)
    k_embs = np.asarray(k_embs, np.float32)
    v_embs = np.asarray(v_embs, np.float32)
    in_maps = []
    for b in range(B):
        m = dict(shared)
        m.update(prep_core(q_embs[b], k_embs[b], v_embs[b]))
        in_maps.append(m)
    res = run_bass_kernel_spmd(nc, in_maps, core_ids=list(range(B)), **run_kwargs)
    out = np.stack([res.results[i]["y"] for i in range(B)], axis=0)
    if run_kwargs:
        kernel.last_results = res
    return out


if __name__ == "__main__":
    rng = np.random.default_rng(0)
    inputs = {
        "q_embs": rng.standard_normal((B, S, C), np.float32),
        "k_embs": rng.standard_normal((B, S, C), np.float32),
        "v_embs": rng.standard_normal((B, S, C), np.float32),
        "Wq": rng.standard_normal((H, C, DK), np.float32) * 0.02,
        "bq": np.zeros((H, DK), np.float32),
        "Wk": rng.standard_normal((H, C, DK), np.float32) * 0.02,
        "bk": np.zeros((H, DK), np.float32),
        "Wv": rng.standard_normal((H, C, DV), np.float32) * 0.02,
        "bv": np.zeros((H, DV), np.float32),
        "Wo": rng.standard_normal((H * DV, C), np.float32) * 0.02,
        "bo": np.zeros((C,), np.float32),
    }
    out = kernel(**inputs)
    print(out.shape, out.dtype)
